# revision 2
# baseline (speedup 1.0000x reference)
"""Causal GQA attention block (RoPE, 32 q-heads / 8 kv-heads, fp32 I/O) on
8 Trainium2 NeuronCores — head-parallel (tensor-parallel) version.

Sharding: each core owns 4 q-heads and their shared kv-head (GQA group).
Every core loads the full x (both batches) and computes q/k/v projections
for its own heads locally — NO collectives at all. Attention runs with
exact causal loop bounds (uniform across cores, since every core sees all
tokens), so only statically-known diagonal k-tiles need masking. The
out-projection produces a per-core partial (its heads' slice of Wo);
the host sums the 8 partials.

On-chip layout is feature-major: host ships x^T and W^T (bf16) so both
matmul operands have the contraction dim on partitions. RoPE pairs are
partition-aligned by permuting Wq/Wk rows per head (evens then odds);
dot products are permutation-invariant since q and k share the layout.

Softmax: scores computed transposed [k, q]; exp on ACT with the 1/8
scale folded in; max-subtraction skipped (|s|/8 is small for these
inputs); the denominator comes from a ones-column appended to v in the
AV matmul; normalization happens once after the full AV accumulation.

v is projected feature-major together with k (same psum tile), drained
to SBUF and flipped token-major with hardware DMA transposes.
"""

import sys
import json

sys.path.insert(0, "/opt/trn_rl_repo")

import numpy as np
import ml_dtypes

import concourse.bass as bass
import concourse.tile as tile
from concourse import mybir

F32 = mybir.dt.float32
BF16 = mybir.dt.bfloat16
BF = ml_dtypes.bfloat16
AF = mybir.ActivationFunctionType

# ---------------------------------------------------------------------------
# walrus workaround: this build supports one semaphore wait per instruction,
# but TileContext's tail drain attaches several. Split the extras onto
# standalone EventSemaphore instructions placed just before the instruction.
# ---------------------------------------------------------------------------


def _fix_multiwait(bir_bytes):
    d = json.loads(bir_bytes)
    ctr = 0
    changed = False
    for fn in d.get("functions", []):
        for blk in fn.get("blocks", []):
            new_insts = []
            for inst in blk["instructions"]:
                si = inst.get("sync_info") or {}
                waits = si.get("on_wait") or []
                if len(waits) > 1:
                    changed = True
                    for w in waits[:-1]:
                        ctr += 1
                        new_insts.append({
                            "debug": inst.get("debug", 0),
                            "engine": inst["engine"],
                            "ins": [],
                            "name": f"mwfix_{ctr}_{inst['name']}",
                            "opcode": "EventSemaphore",
                            "outs": [],
                            "sync_info": {"on_update": [], "on_wait": [w]},
                        })
                    si["on_wait"] = [waits[-1]]
                new_insts.append(inst)
            blk["instructions"] = new_insts
    return json.dumps(d).encode() if changed else bir_bytes


def _install_birfix():
    from concourse import bass_utils, bass2jax

    if getattr(bass_utils, "_mwfix_installed", False):
        return
    orig = bass_utils.compile_bir_kernel

    def patched(bir_json, tmpdir, neff_name="file.neff", **kw):
        if isinstance(bir_json, str):
            bir_json = bir_json.encode()
        return orig(_fix_multiwait(bir_json), tmpdir, neff_name, **kw)

    bass_utils.compile_bir_kernel = patched
    bass_utils._mwfix_installed = True
    bass2jax.compile_bir_kernel = patched


# ---------------------------------------------------------------------------
# configuration
# ---------------------------------------------------------------------------


class Cfg:
    def __init__(self, B=2, T=2048, DIM=2048, NH=32, NKV=8, HD=64,
                 rope_base=10000.0):
        self.B, self.T, self.DIM = B, T, DIM
        self.NH, self.NKV, self.HD = NH, NKV, HD
        self.rope_base = rope_base
        self.NCORES = 8
        self.HQ = NH // self.NCORES      # q heads per core (4)
        self.NCT = DIM // 128            # contraction chunks (16)
        self.TC = 512                    # projection token chunk
        self.NTC = T // self.TC          # 4
        self.QC = 512                    # attention query chunk
        self.NQC = T // self.QC         # 4
        self.KT = 128                    # k-tile (partition dim)
        self.NKT = T // self.KT         # 16
        self.WQKV = self.HQ * HD + 2 * HD  # 384 packed weight cols
        self.VST = 96  # v_sb per-k-tile stride: 64 v cols + ones col,
                       # padded to a 32-element boundary (the DMA-transpose
                       # XBAR corrupts neighbors on unaligned outputs)


FULL = Cfg()


# ---------------------------------------------------------------------------
# device program
# ---------------------------------------------------------------------------


def build_nc(cfg: Cfg, reps=1):
    c = cfg
    nc = bass.Bass(num_devices=c.NCORES)

    BT = c.B * c.T
    xT = nc.declare_dram_parameter("xT", [c.DIM, BT], BF16, isOutput=False)
    wqkvT = nc.declare_dram_parameter("wqkvT", [c.DIM, c.WQKV], BF16,
                                      isOutput=False)
    woT = nc.declare_dram_parameter("woT", [c.HQ * c.HD, c.DIM], BF16,
                                    isOutput=False)
    cosq = nc.declare_dram_parameter("cosq", [c.HD // 2, c.T], BF16,
                                     isOutput=False)
    sinq = nc.declare_dram_parameter("sinq", [c.HD // 2, c.T], BF16,
                                     isOutput=False)
    dmask = nc.declare_dram_parameter("dmask", [c.KT, 384 + c.QC], BF16,
                                      isOutput=False)
    outp = nc.declare_dram_parameter("outp", [BT, c.DIM], BF16, isOutput=True)

    HD2 = c.HD // 2

    with tile.TileContext(nc) as tc:
        with tc.tile_pool(name="persist", bufs=1) as persist:
            # weights + tables, resident for the whole program.
            # x(b0) chunk loads are interleaved with the weight chunk loads
            # so the first projection matmul can start as early as possible.
            wqkv_sb = persist.tile([128, c.NCT * c.WQKV], BF16)
            xT_sb = persist.tile([128, c.NCT * c.T], BF16)
            for ct in range(c.NCT):
                nc.sync.dma_start(
                    wqkv_sb[:, ct * c.WQKV:(ct + 1) * c.WQKV],
                    wqkvT[ct * 128:(ct + 1) * 128, :])
                nc.sync.dma_start(
                    xT_sb[:, ct * c.T:(ct + 1) * c.T],
                    xT[ct * 128:(ct + 1) * 128, 0:c.T])
            wo_sb = persist.tile([128, 2 * c.DIM], BF16)
            for a in range(2):
                nc.sync.dma_start(
                    wo_sb[:, a * c.DIM:(a + 1) * c.DIM],
                    woT[a * 128:(a + 1) * 128, :])
            # cos/sin replicated at all four 32-row partition offsets so
            # every rope tensor_tensor op has its SBUF inputs on the same
            # start partition (walrus checkSBSameStartPartition).
            cos_sb = persist.tile([128, c.T], BF16)
            sin_sb = persist.tile([128, c.T], BF16)
            for rr in range(4):
                nc.sync.dma_start(cos_sb[rr * HD2:(rr + 1) * HD2, :],
                                  cosq[:])
                nc.sync.dma_start(sin_sb[rr * HD2:(rr + 1) * HD2, :],
                                  sinq[:])
            dmask_sb = persist.tile([c.KT, 384 + c.QC], BF16)
            nc.sync.dma_start(dmask_sb[:], dmask[:])
            ones1 = persist.tile([1, c.HD], F32)
            nc.vector.memset(ones1[:], 1.0)
            qT_sb = persist.tile([c.HD, c.HQ * c.T], BF16)
            kT_sb = persist.tile([c.HD, c.T], BF16)
            v_sb = persist.tile([128, c.NKT * c.VST], BF16)
            yT_sb = persist.tile([128, (c.HQ // 2) * c.T], BF16)
            # ones columns of v_sb (never overwritten afterwards)
            nc.gpsimd.memset(v_sb[:], 1.0)

            def xt_chunk(ct, tok0, w):
                return xT_sb[:, ct * c.T + tok0:ct * c.T + tok0 + w]

            def load_x(b):
                for ct in range(c.NCT):
                    nc.sync.dma_start(
                        xT_sb[:, ct * c.T:(ct + 1) * c.T],
                        xT[ct * 128:(ct + 1) * 128, b * c.T:(b + 1) * c.T])

            def rope_tile(eng, dr, r0, t0, dst_ev, dst_od, tmp):
                """dr: bf16-drained psum tile; r0: partition base of the
                head (evens at r0, odds at r0+HD2). Every tensor_tensor
                keeps both SBUF inputs on one start partition: temporaries
                are placed on the partition range of the second operand.
                All-bf16 ops for the DVE 2x 16-bit mode."""
                ev = dr[r0:r0 + HD2, :]
                od = dr[r0 + HD2:r0 + c.HD, :]
                cos_ev = cos_sb[r0:r0 + HD2, t0:t0 + c.TC]
                cos_od = cos_sb[r0 + HD2:r0 + c.HD, t0:t0 + c.TC]
                sin_ev = sin_sb[r0:r0 + HD2, t0:t0 + c.TC]
                sin_od = sin_sb[r0 + HD2:r0 + c.HD, t0:t0 + c.TC]
                t1 = tmp.tile([128, c.TC], BF16, tag="t1")
                t2 = tmp.tile([128, c.TC], BF16, tag="t2")
                # out_ev = ev*cos - od*sin (both temps on od's partitions)
                eng.tensor_mul(t1[r0 + HD2:r0 + c.HD, :], ev, cos_ev)
                eng.tensor_mul(t2[r0 + HD2:r0 + c.HD, :], od, sin_od)
                eng.tensor_sub(dst_ev, t1[r0 + HD2:r0 + c.HD, :],
                               t2[r0 + HD2:r0 + c.HD, :])
                t3 = tmp.tile([128, c.TC], BF16, tag="t3")
                t4 = tmp.tile([128, c.TC], BF16, tag="t4")
                # out_od = ev*sin + od*cos (both temps on ev's partitions)
                eng.tensor_mul(t3[r0:r0 + HD2, :], ev, sin_ev)
                eng.tensor_mul(t4[r0:r0 + HD2, :], od, cos_od)
                eng.tensor_add(dst_od, t3[r0:r0 + HD2, :],
                               t4[r0:r0 + HD2, :])

            # Program-wide pools: a single PSUM layout (4 "av" banks +
            # 2x2 "s" banks shared by proj/scores/norm/oproj) avoids any
            # pool-scope transition stalls between phases.
            with tc.tile_pool(name="avpool", bufs=4, space="PSUM") as avp, \
                 tc.tile_pool(name="spool", bufs=2, space="PSUM") as spool, \
                 tc.tile_pool(name="ropet", bufs=3) as ropet, \
                 tc.tile_pool(name="drpool", bufs=4) as drpool, \
                 tc.tile_pool(name="epool", bufs=4) as epool, \
                 tc.tile_pool(name="npool", bufs=3) as npool, \
                 tc.tile_pool(name="osb", bufs=3) as osb:
              for _rep in range(reps):
                for b in range(c.B):
                    # ------------- q/k/v projections + rope -------------
                    for tci in range(c.NTC):
                        t0 = tci * c.TC
                        for a in range(2):  # q head pairs
                            ps = spool.tile([128, 2 * c.QC], F32, tag="s")
                            for ct in range(c.NCT):
                                nc.tensor.matmul(
                                    ps[:, 0:c.TC],
                                    wqkv_sb[:, ct * c.WQKV + a * 128:
                                            ct * c.WQKV + (a + 1) * 128],
                                    xt_chunk(ct, t0, c.TC),
                                    start=(ct == 0), stop=(ct == c.NCT - 1))
                            dr = drpool.tile([128, c.TC], BF16, tag="dr")
                            nc.scalar.copy(dr[:], ps[:, 0:c.TC])
                            for hh in range(2):
                                h = 2 * a + hh
                                qcol = h * c.T + t0
                                rope_tile(
                                    nc.vector, dr, hh * c.HD, t0,
                                    qT_sb[0:HD2, qcol:qcol + c.TC],
                                    qT_sb[HD2:c.HD, qcol:qcol + c.TC],
                                    ropet)
                        # k (rows 0:64) + v (rows 64:128) in one psum
                        ps = spool.tile([128, 2 * c.QC], F32, tag="s")
                        for ct in range(c.NCT):
                            nc.tensor.matmul(
                                ps[:, 0:c.TC],
                                wqkv_sb[:, ct * c.WQKV + 256:
                                        ct * c.WQKV + c.WQKV],
                                xt_chunk(ct, t0, c.TC),
                                start=(ct == 0), stop=(ct == c.NCT - 1))
                        dr = drpool.tile([128, c.TC], BF16, tag="dr")
                        nc.scalar.copy(dr[:], ps[:, 0:c.TC])
                        rope_tile(nc.vector, dr, 0, t0,
                                  kT_sb[0:HD2, t0:t0 + c.TC],
                                  kT_sb[HD2:c.HD, t0:t0 + c.TC],
                                  ropet)
                        # v rows of the drained tile -> token-major tiles
                        # via DMA transpose (DMA reads any partitions)
                        for gg in range(c.TC // c.KT):
                            g = tci * (c.TC // c.KT) + gg
                            nc.sync.dma_start_transpose(
                                v_sb[:, g * c.VST:g * c.VST + c.HD],
                                dr[c.HD:128, gg * c.KT:(gg + 1) * c.KT])

                    # x for the next batch: emitted here so the loads run
                    # as soon as this batch's projection releases xT_sb
                    # (they overlap the attention phase).
                    if b == 0:
                        load_x(1)
                    elif _rep < reps - 1:
                        load_x(0)

                    # ------- attention + interleaved out projection -------
                    # oproj for query chunk qc runs right after its
                    # normalization, sharing the "s" PSUM tag with the
                    # scores tiles, so the tail after the last chunk is
                    # tiny and stores spread across the whole phase.
                    for qc in range(c.NQC):
                        q0 = qc * c.QC
                        av = [avp.tile([c.HD + 1, c.QC], F32, tag="av",
                                       name=f"av_{_rep}_{b}_{qc}_{h}")
                              for h in range(c.HQ)]
                        ng = 4 * qc + 4
                        for g in range(ng):
                            kt = kT_sb[:, g * c.KT:(g + 1) * c.KT]
                            vt = v_sb[:, g * c.VST:g * c.VST + c.HD + 1]
                            diag = g >= 4 * qc
                            for a in range(2):
                                sps = spool.tile([128, 2 * c.QC], F32,
                                                 tag="s")
                                for hh in range(2):
                                    h = 2 * a + hh
                                    nc.tensor.matmul(
                                        sps[:, hh * c.QC:(hh + 1) * c.QC],
                                        kt,
                                        qT_sb[:, h * c.T + q0:
                                              h * c.T + q0 + c.QC],
                                        start=True, stop=True)
                                ex = epool.tile([128, 2 * c.QC], BF16,
                                                tag="ex")
                                nc.scalar.activation(
                                    ex[:], sps[:], AF.Exp, bias=0.0,
                                    scale=float(1.0 / np.sqrt(c.HD)))
                                if diag:
                                    d = g - 4 * qc
                                    mk = dmask_sb[:, 384 - 128 * d:
                                                  384 - 128 * d + c.QC]
                                    exm = epool.tile([128, 2 * c.QC], BF16,
                                                     tag="exm")
                                    nc.gpsimd.tensor_mul(
                                        exm[:, 0:c.QC], ex[:, 0:c.QC], mk)
                                    nc.vector.tensor_mul(
                                        exm[:, c.QC:2 * c.QC],
                                        ex[:, c.QC:2 * c.QC], mk)
                                else:
                                    exm = ex
                                for hh in range(2):
                                    nc.tensor.matmul(
                                        av[2 * a + hh][:],
                                        vt,
                                        exm[:, hh * c.QC:(hh + 1) * c.QC],
                                        start=(g == 0), stop=(g == ng - 1),
                                        skip_group_check=True)
                        for h in range(c.HQ):
                            # (PSUM reads must stay off gpsimd)
                            l_sb = npool.tile([1, c.QC], F32, tag="l")
                            nc.vector.tensor_copy(l_sb[:],
                                                  av[h][c.HD:c.HD + 1, :])
                            bc_ps = spool.tile([128, 2 * c.QC], F32,
                                               tag="s")
                            nc.tensor.matmul(bc_ps[0:c.HD, 0:c.QC],
                                             ones1[:], l_sb[:],
                                             start=True, stop=True)
                            bc = npool.tile([c.HD, c.QC], F32, tag="bc")
                            nc.vector.reciprocal(bc[:],
                                                 bc_ps[0:c.HD, 0:c.QC])
                            ycol = (h // 2) * c.T + q0
                            nc.vector.tensor_mul(
                                yT_sb[(h % 2) * c.HD:(h % 2 + 1) * c.HD,
                                      ycol:ycol + c.QC],
                                av[h][0:c.HD, :], bc[:])
                        # out projection for this chunk's tokens
                        for tt in range(4 * qc, 4 * qc + 4):
                            o_sb = osb.tile([128, c.DIM], BF16, tag="ot")
                            for half in range(2):
                                ops = spool.tile([128, 2 * c.QC], F32,
                                                 tag="s")
                                for a2 in range(2):
                                    lhs = yT_sb[:, a2 * c.T + tt * 128:
                                                a2 * c.T + (tt + 1) * 128]
                                    for oo in range(2):
                                        oc = 2 * half + oo
                                        nc.tensor.matmul(
                                            ops[:, oo * 512:
                                                (oo + 1) * 512],
                                            lhs,
                                            wo_sb[:, a2 * c.DIM + oc * 512:
                                                  a2 * c.DIM +
                                                  (oc + 1) * 512],
                                            start=(a2 == 0),
                                            stop=(a2 == 1))
                                nc.scalar.copy(
                                    o_sb[:, half * 1024:(half + 1) * 1024],
                                    ops[:])
                            nc.sync.dma_start(
                                outp[b * c.T + tt * 128:
                                     b * c.T + (tt + 1) * 128, :], o_sb[:])

    return nc


# ---------------------------------------------------------------------------
# host side
# ---------------------------------------------------------------------------


def _rope_perm(n_heads, hd):
    p = []
    for h in range(n_heads):
        p.extend(h * hd + np.arange(0, hd, 2))
        p.extend(h * hd + np.arange(1, hd, 2))
    return np.array(p)


def _cos_sin(T, hd, base):
    inv = 1.0 / base ** (np.arange(0, hd, 2, dtype=np.float64) / hd)
    fr = np.outer(inv, np.arange(T, dtype=np.float64))
    return np.cos(fr).astype(np.float32), np.sin(fr).astype(np.float32)


def make_inputs(cfg: Cfg, x, Wq, Wk, Wv, Wo):
    c = cfg
    B, T, D = c.B, c.T, c.DIM
    xT = np.ascontiguousarray(
        x.reshape(B * T, D).T.astype(BF))  # [D, B*T]
    cos, sin = _cos_sin(T, c.HD, c.rope_base)
    cos, sin = cos.astype(BF), sin.astype(BF)
    kk = np.arange(c.KT)
    cc = np.arange(384 + c.QC)
    dm = (kk[:, None] <= (cc[None, :] - 384)).astype(BF)

    permq_head = np.concatenate([np.arange(0, c.HD, 2),
                                 np.arange(1, c.HD, 2)])
    in_maps = []
    for core in range(c.NCORES):
        h0 = core * c.HQ
        qrows = np.concatenate(
            [(h0 + h) * c.HD + permq_head for h in range(c.HQ)])
        krows = core * c.HD + permq_head
        vrows = core * c.HD + np.arange(c.HD)
        wq = Wq[qrows]                      # [256, D]
        wk = Wk[krows]                      # [64, D]
        wv = Wv[vrows]                      # [64, D]
        wqkvT = np.ascontiguousarray(
            np.concatenate([wq, wk, wv], axis=0).T.astype(BF))  # [D, 384]
        woT = np.ascontiguousarray(
            Wo[:, core * c.HQ * c.HD:(core + 1) * c.HQ * c.HD].T.astype(BF))
        in_maps.append({
            "xT": xT, "wqkvT": wqkvT, "woT": woT,
            "cosq": cos, "sinq": sin, "dmask": dm,
        })
    return in_maps


def assemble(cfg: Cfg, results):
    c = cfg
    acc = results[0]["outp"].astype(np.float32)
    for core in range(1, c.NCORES):
        acc += results[core]["outp"].astype(np.float32)
    return acc.reshape(c.B, c.T, c.DIM)


_CACHE = {}


def kernel(x, Wq, Wk, Wv, Wo):
    _install_birfix()
    import os
    from concourse.bass_utils import run_bass_kernel_spmd

    cfg = FULL
    if "nc" not in _CACHE:
        _CACHE["nc"] = build_nc(cfg)
    nc = _CACHE["nc"]
    in_maps = make_inputs(cfg, np.asarray(x), np.asarray(Wq), np.asarray(Wk),
                          np.asarray(Wv), np.asarray(Wo))
    try:
        res = run_bass_kernel_spmd(nc, in_maps,
                                   core_ids=list(range(cfg.NCORES)))
    except ModuleNotFoundError:
        os.environ["BASS_NEVER_TRACE"] = "1"
        res = run_bass_kernel_spmd(nc, in_maps,
                                   core_ids=list(range(cfg.NCORES)))
    return assemble(cfg, res.results)
